# revision 1
# baseline (speedup 1.0000x reference)
"""Trainium2 Bass kernel for nn_DecodingLoss_BCEBased (segment_reduce).

Strategy (data-parallel over batch, 8 NeuronCores, 128 batch rows/core):
  - Host pre-transposes llrs to token-major [N, B] so each core DMAs its
    batch slice directly into a token-stripe SBUF layout (token n lives at
    partition n%128, stripe n//128, 128 bf16 batch values = 256B payload).
  - t = tanh(0.5*llrs) computed on ScalarE (ACT), stored bf16.
  - Check supports are gathered with SBUF-source dma_gather(transpose=True):
    out[b, i] = t[cols_flat[i], b] lands batch-on-partition.
  - BCEWithLogits simplifies exactly: softplus(z) - z*y with
    z = -2*arctanh(p) equals log2 - log(1 - s*p), s = 2y-1. So per check:
    product-of-8 (DVE mult tree, bf16->f32), clip, multiply by host-prepared
    sign tensor, then one ACT Ln(1 - x) with accum_out doing the
    sum-over-checks reduction for free.
  - Observables (8 x 200) go through the same path, padded to 256 with a
    dedicated all-ones token so a pow2 mult tree works.
  - Each core returns per-row partial sums S_b = sum ln(1-s*p); the host
    finishes: loss = 0.5*(M+K)*log2 - 0.5*mean(S).
"""
import numpy as np
import concourse.bass as bass
import concourse.tile as tile
from concourse import bacc, mybir
from concourse.bass_utils import run_bass_kernel_spmd

F32 = mybir.dt.float32
BF16 = mybir.dt.bfloat16
I16 = mybir.dt.int16
AF = mybir.ActivationFunctionType
ALU = mybir.AluOpType

P = 128            # batch rows per core == SBUF partitions
N_CORES = 8
B, N, M, K = 1024, 20000, 10000, 8
CHK_W, OBS_W = 8, 200
EPS = 1e-6

N_TOK_PAD = ((N + P - 1) // P + 1) * P     # extra stripe for the ones-token
CHK_CHUNK = 512
N_CHK_PAD = ((M + CHK_CHUNK - 1) // CHK_CHUNK) * CHK_CHUNK
OBS_PW = 256                                # next pow2 >= OBS_W

_NC_CACHE = {}
_TRACE = False  # test.py flips this to get neuron-profile exec_time_ns


def _build_kernel():
    n_stripe = N_TOK_PAD // P
    n_chunk = N_CHK_PAD // CHK_CHUNK
    gidx = CHK_CHUNK * 8
    n_obs_idx = K * OBS_PW

    nc = bacc.Bacc("TRN2", target_bir_lowering=False, debug=False,
                   num_devices=N_CORES)

    llrsT = nc.dram_tensor("llrsT", [N_TOK_PAD, P], F32, kind="ExternalInput").ap()
    sgn = nc.dram_tensor("sgn", [P, N_CHK_PAD], F32, kind="ExternalInput").ap()
    sgn_obs = nc.dram_tensor("sgn_obs", [P, K], F32, kind="ExternalInput").ap()
    chk_idx = nc.dram_tensor(
        "chk_idx", [P, N_CHK_PAD * 8 // 16], I16, kind="ExternalInput").ap()
    obs_idx = nc.dram_tensor(
        "obs_idx", [P, n_obs_idx // 16], I16, kind="ExternalInput").ap()
    out = nc.dram_tensor("out", [P, 1], F32, kind="ExternalOutput").ap()

    with tile.TileContext(nc) as tc:
        with (
            tc.tile_pool(name="tok", bufs=1) as tok_pool,
            tc.tile_pool(name="stage", bufs=3) as stage_pool,
            tc.tile_pool(name="idx", bufs=1) as idx_pool,
            tc.tile_pool(name="g", bufs=3) as g_pool,
            tc.tile_pool(name="prod", bufs=2) as prod_pool,
            tc.tile_pool(name="sg", bufs=2) as sg_pool,
            tc.tile_pool(name="acc", bufs=1) as acc_pool,
        ):
            # token tile: t = tanh(0.5*llrs), bf16, token-stripe layout
            tokT = tok_pool.tile([P, N_TOK_PAD], BF16)
            r = 0
            while r < n_stripe:
                ns = min(16, n_stripe - r)
                st = stage_pool.tile([P, 16 * P], F32, tag="stage")
                src = llrsT[bass.ds(r * P, ns * P), :].rearrange(
                    "(rr p) b -> p rr b", p=P)
                dst = st[:, : ns * P].rearrange("p (rr b) -> p rr b", b=P)
                nc.sync.dma_start(dst, src)
                nc.scalar.activation(
                    tokT[:, bass.ds(r * P, ns * P)], st[:, : ns * P], AF.Tanh,
                    scale=0.5)
                r += ns

            # last (padding) stripe = exactly 1.0: ones-tokens for obs padding
            nc.vector.memset(tokT[:, bass.ds((n_stripe - 1) * P, P)], 1.0)

            chk_idx_t = idx_pool.tile([P, N_CHK_PAD * 8 // 16], I16)
            nc.sync.dma_start(chk_idx_t[:], chk_idx)
            obs_idx_t = idx_pool.tile([P, n_obs_idx // 16], I16)
            nc.sync.dma_start(obs_idx_t[:], obs_idx)

            acc = acc_pool.tile([P, n_chunk + 2], F32)

            # clamp constant tile: tensor_scalar is pathologically slow on
            # this path (~39us per [128,1024] op), tensor_tensor(min) is not
            kmax = acc_pool.tile([P, CHK_CHUNK], F32)
            nc.vector.memset(kmax[:], 1.0 - EPS)

            def gather(dst_tile, idxs_ap, n_idx):
                nc.gpsimd.dma_gather(
                    out_ap=dst_tile[:].rearrange("p (one i) -> p one i", one=1),
                    in_ap=tokT[:],
                    idxs_ap=idxs_ap,
                    num_idxs=n_idx,
                    num_idxs_reg=n_idx,
                    elem_size=P,
                    transpose=True,
                    single_packet=False,
                    sbuf_tokens_per_rank=P,
                    sbuf_free_dim_per_rank=P * 2,
                    sbuf_free_dim_pad_per_rank=0,
                    sbuf_byte_offset=0,
                )

            # observables
            gob = g_pool.tile([P, n_obs_idx], BF16, tag="gob")
            gather(gob, obs_idx_t[:], n_obs_idx)
            cur = gob[:].rearrange("p (k w) -> p k w", w=OBS_PW)
            w = OBS_PW
            lvl = 0
            while w > 2:
                nxt_t = prod_pool.tile([P, K * w // 2], BF16, tag=f"ob{lvl % 2}")
                nxt = nxt_t[:].rearrange("p (k w) -> p k w", w=w // 2)
                nc.vector.tensor_tensor(nxt, cur[:, :, 0::2], cur[:, :, 1::2],
                                        ALU.mult)
                cur = nxt
                w //= 2
                lvl += 1
            pob = prod_pool.tile([P, K], F32, tag="pob")
            nc.vector.tensor_tensor(pob[:], cur[:, :, 0], cur[:, :, 1], ALU.mult)
            sgo = sg_pool.tile([P, K], F32, tag="sgo")
            nc.sync.dma_start(sgo[:], sgn_obs)
            nc.vector.tensor_tensor(pob[:], pob[:], sgo[:], ALU.mult)
            nc.vector.tensor_tensor(pob[:], pob[:], kmax[:, :K], ALU.min)
            lno = sg_pool.tile([P, K], F32, tag="lno")
            nc.scalar.activation(
                lno[:], pob[:], AF.Ln, bias=1.0, scale=-1.0,
                accum_out=acc[:, n_chunk: n_chunk + 1])
            nc.vector.memset(acc[:, n_chunk + 1: n_chunk + 2], 0.0)

            for c in range(n_chunk):
                g = g_pool.tile([P, gidx], BF16, tag="g")
                gather(g, chk_idx_t[:, bass.ds(c * gidx // 16, gidx // 16)], gidx)
                g3 = g[:].rearrange("p (m w) -> p m w", w=8)
                p1 = prod_pool.tile([P, CHK_CHUNK * 4], BF16, tag="p1")
                p13 = p1[:].rearrange("p (m w) -> p m w", w=4)
                nc.vector.tensor_tensor(p13, g3[:, :, 0::2], g3[:, :, 1::2],
                                        ALU.mult)
                p2 = prod_pool.tile([P, CHK_CHUNK * 2], BF16, tag="p2")
                p23 = p2[:].rearrange("p (m w) -> p m w", w=2)
                nc.vector.tensor_tensor(p23, p13[:, :, 0::2], p13[:, :, 1::2],
                                        ALU.mult)
                pf = prod_pool.tile([P, CHK_CHUNK], F32, tag="pf")
                nc.vector.tensor_tensor(pf[:], p23[:, :, 0], p23[:, :, 1],
                                        ALU.mult)
                sg = sg_pool.tile([P, CHK_CHUNK], F32, tag="sg")
                nc.sync.dma_start(sg[:], sgn[:, bass.ds(c * CHK_CHUNK, CHK_CHUNK)])
                sp = sg_pool.tile([P, CHK_CHUNK], F32, tag="sp")
                nc.vector.tensor_tensor(sp[:], pf[:], sg[:], ALU.mult)
                # clamp s*p <= 1-eps (== reference's two-sided clip of p)
                spc = sg_pool.tile([P, CHK_CHUNK], F32, tag="spc")
                nc.vector.tensor_tensor(spc[:], sp[:], kmax[:], ALU.min)
                lnd = sg_pool.tile([P, CHK_CHUNK], F32, tag="lnd")
                nc.scalar.activation(
                    lnd[:], spc[:], AF.Ln, bias=1.0, scale=-1.0,
                    accum_out=acc[:, c: c + 1])

            s_t = acc_pool.tile([P, 1], F32)
            nc.vector.tensor_reduce(s_t[:], acc[:], mybir.AxisListType.X, ALU.add)
            nc.sync.dma_start(out, s_t[:])

    nc.compile()
    return nc


def _get_nc():
    if "nc" not in _NC_CACHE:
        _NC_CACHE["nc"] = _build_kernel()
    return _NC_CACHE["nc"]


def _wrap_idx(flat):
    # dma_gather index layout: unwrapped[s*16+p] = tile[p, s], replicated
    # across the eight 16-partition groups
    n = flat.shape[0]
    w = flat.reshape(n // 16, 16).T.astype(np.int16)
    return np.tile(w, (8, 1))


def kernel(llrs, syndromes, observables, chk_cols, obs_cols):
    llrs = np.asarray(llrs, dtype=np.float32)
    syndromes = np.asarray(syndromes, dtype=np.float32)
    observables = np.asarray(observables, dtype=np.float32)
    chk_cols = np.asarray(chk_cols)
    obs_cols = np.asarray(obs_cols)

    nc = _get_nc()

    llrsT = np.zeros((N_TOK_PAD, B), np.float32)
    llrsT[:N] = np.ascontiguousarray(llrs.T)
    sgn = np.zeros((B, N_CHK_PAD), np.float32)
    sgn[:, :M] = 2.0 * syndromes - 1.0         # s = 2y-1; padding stays 0
    sgn_obs = (2.0 * observables - 1.0).astype(np.float32)

    chk_flat = np.zeros((N_CHK_PAD, 8), np.int64)
    chk_flat[:M] = chk_cols
    chk_idx = _wrap_idx(chk_flat.reshape(-1))
    ones_id = N_TOK_PAD - 1                    # any token in the all-ones stripe
    obs_flat = np.full((K, OBS_PW), ones_id, np.int64)
    obs_flat[:, :OBS_W] = obs_cols
    obs_idx = _wrap_idx(obs_flat.reshape(-1))

    in_maps = []
    for c in range(N_CORES):
        sl = slice(c * P, (c + 1) * P)
        in_maps.append({
            "llrsT": np.ascontiguousarray(llrsT[:, sl]),
            "sgn": np.ascontiguousarray(sgn[sl]),
            "sgn_obs": np.ascontiguousarray(sgn_obs[sl]),
            "chk_idx": chk_idx,
            "obs_idx": obs_idx,
        })

    res = run_bass_kernel_spmd(nc, in_maps, core_ids=list(range(N_CORES)),
                               trace=_TRACE)
    _NC_CACHE["exec_time_ns"] = res.exec_time_ns
    S = np.concatenate([r["out"][:, 0] for r in res.results])
    loss_b = 0.5 * (M + K) * np.log(2.0) - 0.5 * S.astype(np.float64)
    return np.float32(loss_b.mean())



# revision 4
# speedup vs baseline: 3.1550x; 3.1550x over previous
"""Trainium2 Bass kernel for nn_DecodingLoss_BCEBased (segment_reduce).

Strategy v2 (4 batch-groups x 2 check-halves over 8 NeuronCores):
  - Each core covers 256 batch rows (two 128-row blocks j=0,1 packed into
    one 512B token row) and half the checks (5120 incl. pad).
  - Token table in SBUF: tokT[p, stripe*256 + j*128 + b] = tanh(0.5*llr),
    bf16, so each gather descriptor moves 512B (vs 256B at 128 batch/core)
    -> half the descriptors for the same bytes.
  - dma_gather's descriptor generation runs on ONE Q7 core-pair selected by
    queue_num (ucode: cpu_id/2 == queue_num). The v1 kernel put all gathers
    on queue 0, serializing ~38us/chunk on cores 0+1 while cores 2-7 idled;
    that cadence was the whole 750us wall time. v2 builds with
    num_swdge_queues=4 and rotates gathers across queues 0-3 so four
    core-pairs generate descriptors concurrently.
  - Gather idx order is slot-major per chunk (slot s of all 512 checks,
    then slot s+1 ...) so the product-of-8 tree is three unit-stride
    contiguous bf16 multiplies (v1's strided 0::2/1::2 reads ran ~2.5x
    slower than contiguous).
  - BCEWithLogits identity: softplus(z) - z*y with z = -2*arctanh(p)
    equals log2 - log(1 - s*p), s = 2y-1. Per check: tree product, * sgn
    (bf16), clamp <= 1-2^-8, one ACT Ln(1-x) per j with accum_out doing
    the sum-over-checks reduction.
  - Observables (8 x 200, padded to 256 with an all-ones token) use the
    same path on every core; half-0 cores get sgn_obs=0 so they contribute
    nothing (keeps the 8 cores' work identical).
  - Each core returns S[p, j] = sum ln(1-s*p) over its check half; host:
    loss = 0.5*(M+K)*log2 - 0.5*mean_b(S_b).
"""
import numpy as np
import ml_dtypes
import concourse.bass as bass
import concourse.tile as tile
from concourse import bacc, mybir
from concourse.bass_utils import run_bass_kernel_spmd

F32 = mybir.dt.float32
BF16 = mybir.dt.bfloat16
I16 = mybir.dt.int16
AF = mybir.ActivationFunctionType
ALU = mybir.AluOpType
BF = ml_dtypes.bfloat16

P = 128            # SBUF partitions
N_CORES = 8
B, N, M, K = 1024, 20000, 10000, 8
CHK_W, OBS_W = 8, 200

NBG = 4            # batch groups (256 rows each)
NJ = 2             # 128-row blocks per core
BW = NJ * P        # batch rows per core = 256
HALF = M // 2      # checks per half (5000)
CHK_CHUNK = 512
N_CHK_HALF = 5120  # padded checks per core (10 chunks)
N_CHUNKS = N_CHK_HALF // CHK_CHUNK
OBS_PW = 256       # obs support padded to pow2

N_STRIPE = (N + P - 1) // P          # 157 data stripes
ONES_ID = N_STRIPE * P               # token in the all-ones stripe
N_TOK_PAD = N_STRIPE * P             # 20096 (dram rows; ones stripe is SBUF-only)
TOK_ELEMS = (N_STRIPE + 1) * BW      # table free elems per partition (bf16)

GIDX = CHK_CHUNK * CHK_W             # 4096 idx per chunk gather
N_OBS_IDX = K * OBS_PW               # 2048

KMAX = 1.0 - 2.0 ** -8               # clamp, exactly representable in bf16

_NC_CACHE = {}
_TRACE = False  # test.py flips this to get neuron-profile exec_time_ns


def _build_kernel():
    nc = bacc.Bacc("TRN2", target_bir_lowering=False, debug=False,
                   num_devices=N_CORES, num_swdge_queues=4)

    llrsT2 = nc.dram_tensor("llrsT2", [N_TOK_PAD, BW], BF16,
                            kind="ExternalInput").ap()
    sgn = nc.dram_tensor("sgn", [P, NJ * N_CHK_HALF], BF16,
                         kind="ExternalInput").ap()
    sgn_obs = nc.dram_tensor("sgn_obs", [P, NJ * K], BF16,
                             kind="ExternalInput").ap()
    chk_idx = nc.dram_tensor(
        "chk_idx", [P, N_CHK_HALF * CHK_W // 16], I16, kind="ExternalInput").ap()
    obs_idx = nc.dram_tensor(
        "obs_idx", [P, N_OBS_IDX // 16], I16, kind="ExternalInput").ap()
    out = nc.dram_tensor("out", [P, NJ], F32, kind="ExternalOutput").ap()

    with tile.TileContext(nc) as tc:
        with (
            tc.tile_pool(name="tok", bufs=1) as tok_pool,
            tc.tile_pool(name="stage", bufs=2) as stage_pool,
            tc.tile_pool(name="idx", bufs=1) as idx_pool,
            tc.tile_pool(name="g", bufs=3) as g_pool,
            tc.tile_pool(name="gob", bufs=1) as gob_pool,
            tc.tile_pool(name="tree", bufs=1) as tree_pool,
            tc.tile_pool(name="sg", bufs=2) as sg_pool,
            tc.tile_pool(name="spc", bufs=2) as spc_pool,
            tc.tile_pool(name="acc", bufs=1) as acc_pool,
        ):
            chk_idx_t = idx_pool.tile([P, N_CHK_HALF * CHK_W // 16], I16,
                                      tag="ichk")
            nc.sync.dma_start(chk_idx_t[:], chk_idx)
            obs_idx_t = idx_pool.tile([P, N_OBS_IDX // 16], I16, tag="iobs")
            nc.sync.dma_start(obs_idx_t[:], obs_idx)
            sgo = idx_pool.tile([P, NJ * K], BF16, tag="sgo")
            nc.sync.dma_start(sgo[:], sgn_obs)

            acc = acc_pool.tile([P, NJ * (N_CHUNKS + 1)], F32, tag="acc")
            kmax = acc_pool.tile([P, NJ * CHK_CHUNK], BF16, tag="kmax")
            nc.vector.memset(kmax[:], KMAX)

            # token table: t = tanh(0.5*llrs), bf16, 512B per token row
            tokT = tok_pool.tile([P, TOK_ELEMS], BF16)
            r = 0
            while r < N_STRIPE:
                ns = min(16, N_STRIPE - r)
                st = stage_pool.tile([P, 16 * BW], BF16, tag="stage")
                src = llrsT2[bass.ds(r * P, ns * P), :].rearrange(
                    "(rr p) b -> p rr b", p=P)
                dst = st[:, : ns * BW].rearrange("p (rr b) -> p rr b", b=BW)
                nc.sync.dma_start(dst, src)
                nc.scalar.activation(
                    tokT[:, bass.ds(r * BW, ns * BW)], st[:, : ns * BW],
                    AF.Tanh, scale=0.5)
                r += ns
            # ones stripe for obs padding
            nc.vector.memset(tokT[:, bass.ds(N_STRIPE * BW, BW)], 1.0)

            def gather(dst3d, idxs_ap, n_idx, q):
                nc.gpsimd.dma_gather(
                    out_ap=dst3d,
                    in_ap=tokT[:],
                    idxs_ap=idxs_ap,
                    num_idxs=n_idx,
                    num_idxs_reg=n_idx,
                    elem_size=BW,            # 256 bf16 = 512B per idx
                    transpose=True,
                    single_packet=False,
                    sbuf_tokens_per_rank=P,
                    sbuf_free_dim_per_rank=BW * 2,
                    sbuf_free_dim_pad_per_rank=0,
                    sbuf_byte_offset=0,
                    queue_num=q,
                )

            # observables: slot-major (8 obs contiguous per slot), 256 slots
            gob = gob_pool.tile([P, NJ * N_OBS_IDX], BF16, tag="gob")
            gather(gob[:].rearrange("p (j i) -> p j i", j=NJ),
                   obs_idx_t[:], N_OBS_IDX, 3)
            cur = gob[:].rearrange("p (j i) -> p j i", j=NJ)
            w = N_OBS_IDX
            lvl = 0
            while w > 2 * K:
                nxt_t = tree_pool.tile([P, NJ * w // 2], BF16, tag=f"ob{lvl}")
                nxt = nxt_t[:].rearrange("p (j i) -> p j i", j=NJ)
                nc.vector.tensor_tensor(nxt, cur[:, :, : w // 2],
                                        cur[:, :, w // 2:], ALU.mult)
                cur = nxt
                w //= 2
                lvl += 1
            pob = tree_pool.tile([P, NJ * K], BF16, tag="pob")
            pob3 = pob[:].rearrange("p (j i) -> p j i", j=NJ)
            nc.vector.tensor_tensor(pob3, cur[:, :, :K], cur[:, :, K:],
                                    ALU.mult)
            nc.vector.tensor_tensor(pob[:], pob[:], sgo[:], ALU.mult)
            nc.vector.tensor_tensor(pob[:], pob[:], kmax[:, : NJ * K], ALU.min)
            lno = tree_pool.tile([P, NJ * K], BF16, tag="lno")
            for j in range(NJ):
                nc.scalar.activation(
                    lno[:, bass.ds(j * K, K)], pob[:, bass.ds(j * K, K)],
                    AF.Ln, bias=1.0, scale=-1.0,
                    accum_out=acc[:, bass.ds(NJ * N_CHUNKS + j, 1)])

            # check chunks: slot-major gather -> contiguous mult tree
            for c in range(N_CHUNKS):
                g = g_pool.tile([P, NJ * GIDX], BF16, tag="g")
                g3 = g[:].rearrange("p (j i) -> p j i", j=NJ)
                gather(g3, chk_idx_t[:, bass.ds(c * GIDX // 16, GIDX // 16)],
                       GIDX, c % 4)
                p1 = tree_pool.tile([P, NJ * GIDX // 2], BF16, tag="p1")
                p13 = p1[:].rearrange("p (j i) -> p j i", j=NJ)
                nc.vector.tensor_tensor(p13, g3[:, :, : GIDX // 2],
                                        g3[:, :, GIDX // 2:], ALU.mult)
                p2 = tree_pool.tile([P, NJ * GIDX // 4], BF16, tag="p2")
                p23 = p2[:].rearrange("p (j i) -> p j i", j=NJ)
                nc.vector.tensor_tensor(p23, p13[:, :, : GIDX // 4],
                                        p13[:, :, GIDX // 4:], ALU.mult)
                p3 = tree_pool.tile([P, NJ * CHK_CHUNK], BF16, tag="p3")
                p33 = p3[:].rearrange("p (j i) -> p j i", j=NJ)
                nc.vector.tensor_tensor(p33, p23[:, :, :CHK_CHUNK],
                                        p23[:, :, CHK_CHUNK:], ALU.mult)
                sg = sg_pool.tile([P, NJ * CHK_CHUNK], BF16, tag="sg")
                nc.sync.dma_start(
                    sg[:], sgn[:, bass.ds(c * NJ * CHK_CHUNK, NJ * CHK_CHUNK)])
                sp = sg_pool.tile([P, NJ * CHK_CHUNK], BF16, tag="sp")
                nc.vector.tensor_tensor(sp[:], p3[:], sg[:], ALU.mult)
                spc = spc_pool.tile([P, NJ * CHK_CHUNK], BF16, tag="spc")
                nc.vector.tensor_tensor(spc[:], sp[:], kmax[:], ALU.min)
                lnd = tree_pool.tile([P, NJ * CHK_CHUNK], BF16, tag="lnd")
                for j in range(NJ):
                    nc.scalar.activation(
                        lnd[:, bass.ds(j * CHK_CHUNK, CHK_CHUNK)],
                        spc[:, bass.ds(j * CHK_CHUNK, CHK_CHUNK)],
                        AF.Ln, bias=1.0, scale=-1.0,
                        accum_out=acc[:, bass.ds(c * NJ + j, 1)])

            s_t = acc_pool.tile([P, NJ], F32, tag="st")
            accv = acc[:].rearrange("p (c j) -> p j c", j=NJ)
            nc.vector.tensor_reduce(s_t[:], accv, mybir.AxisListType.X,
                                    ALU.add)
            nc.sync.dma_start(out, s_t[:])

    nc.compile()
    return nc


def _get_nc():
    if "nc" not in _NC_CACHE:
        _NC_CACHE["nc"] = _build_kernel()
    return _NC_CACHE["nc"]


def _wrap_idx(flat):
    # dma_gather index layout: unwrapped[s*16+p] = tile[p, s], replicated
    # across the eight 16-partition groups
    n = flat.shape[0]
    w = flat.reshape(n // 16, 16).T.astype(np.int16)
    return np.tile(w, (8, 1))


def kernel(llrs, syndromes, observables, chk_cols, obs_cols):
    llrs = np.asarray(llrs, dtype=np.float32)
    syndromes = np.asarray(syndromes, dtype=np.float32)
    observables = np.asarray(observables, dtype=np.float32)
    chk_cols = np.asarray(chk_cols)
    obs_cols = np.asarray(obs_cols)

    nc = _get_nc()

    # token-major llrs, bf16: [N_TOK_PAD, B]
    llrsT = np.zeros((N_TOK_PAD, B), BF)
    llrsT[:N] = llrs.T

    # sgn, padded to N_CHK_HALF per half, laid out [p, c, j, i]
    sgn_full = np.zeros((B, 2 * N_CHK_HALF), BF)
    sgn_full[:, :M] = (2.0 * syndromes - 1.0)
    sgn_obs_full = (2.0 * observables - 1.0).astype(BF)

    # check idx, slot-major per 512-chunk: idx[c*4096 + s*512 + i]
    chk_pad = np.zeros((2 * N_CHK_HALF, CHK_W), np.int64)
    chk_pad[:M] = chk_cols

    def chk_idx_half(h):
        cc = chk_pad[h * N_CHK_HALF:(h + 1) * N_CHK_HALF]
        cc = cc.reshape(N_CHUNKS, CHK_CHUNK, CHK_W).transpose(0, 2, 1)
        return _wrap_idx(cc.reshape(-1))

    chk_idx_w = [chk_idx_half(0), chk_idx_half(1)]

    # obs idx, slot-major: idx[s*8 + k], slots >= 200 -> ones token
    op = np.full((K, OBS_PW), ONES_ID, np.int64)
    op[:, :OBS_W] = obs_cols
    obs_idx_w = _wrap_idx(op.T.reshape(-1))

    in_maps = []
    for core in range(N_CORES):
        bg, half = core // 2, core % 2
        bsl = slice(bg * BW, (bg + 1) * BW)
        # sgn slice -> [p, c, j, i] -> [128, NJ*N_CHK_HALF]
        v = sgn_full[bsl, half * N_CHK_HALF:(half + 1) * N_CHK_HALF]
        v = v.reshape(NJ, P, N_CHUNKS, CHK_CHUNK).transpose(1, 2, 0, 3)
        so = sgn_obs_full[bsl].reshape(NJ, P, K).transpose(1, 0, 2)
        if half == 0:
            so = np.zeros_like(so)
        in_maps.append({
            "llrsT2": np.ascontiguousarray(llrsT[:, bsl]),
            "sgn": np.ascontiguousarray(v.reshape(P, NJ * N_CHK_HALF)),
            "sgn_obs": np.ascontiguousarray(so.reshape(P, NJ * K)),
            "chk_idx": chk_idx_w[half],
            "obs_idx": obs_idx_w,
        })

    res = run_bass_kernel_spmd(nc, in_maps, core_ids=list(range(N_CORES)),
                               trace=_TRACE)
    _NC_CACHE["exec_time_ns"] = res.exec_time_ns
    # S[bg*256 + j*128 + p] = sum over both halves
    S = np.zeros((NBG, NJ, P), np.float64)
    for core in range(N_CORES):
        bg = core // 2
        o = res.results[core]["out"].astype(np.float64)  # [p, j]
        S[bg] += o.T
    S = S.reshape(B)
    loss_b = 0.5 * (M + K) * np.log(2.0) - 0.5 * S
    return np.float32(loss_b.mean())
